# revision 55
# baseline (speedup 1.0000x reference)
"""Trainium2 Bass kernel for nn_MPCActor: MLP (256->512->512->8-useful-cols,
relu/relu/sigmoid) followed by the closed-form equivalent of 100 SGD steps on
u (u_N = A*u0 - 0.5*(p/q)*(1-A), A = (1-2*lr*q)^100).

Data parallel over 8 NeuronCores: batch 32768 -> 4096 rows per core, weights
replicated. All matmuls run in fp8(e4m3) with perf_mode=DoubleRow (K=256 per
instruction, 2x FLOP throughput at the same 1 col/cycle streaming rate). The
obs transpose, weight layouts, fp8 casts and scale folding are done on the
host:
  obsT = obs.T/8 (fp8)      W1h = 8*W1 (fp8)   -> psum1 = z1 exactly
  y1   = relu(z1+b1) (fp8)  W2h = 64*W2 (fp8)  -> psum2 = 64*z2
  y2'  = relu(psum2+64*b2) = 64*y2 (fp8)
  W3h  = 16*W3[:, useful] padded to 16 cols    -> psum3 = 1024*z3
  qpT  = sigmoid(psum3/1024 + b3)  (ACT, feature-major)

PSUM drains are 1 elem/cycle on ACT/DVE (and DMA has no PSUM route), so PSUM
y-tiles span two banks ([128,2,512]) and each drain moves 128x1024 values in
one instruction; three such groups rotate so both drain engines stay busy
while the PE fills the third. The pair's two z3 results share one PSUM bank
at partition offsets 0/32 (concurrent col-groups via tile_position), the
transposed qp lands in one more bank, and the closed-form runs per pair on
the otherwise idle GPSIMD. A short burst of dummy matmuls at t=0 keeps the
PE busy through the HAM warm-up window so real matmuls run at 2.4 GHz.
"""

import numpy as np
import ml_dtypes

import concourse.bass as bass
import concourse.mybir as mybir
import concourse.tile as tile
from concourse import bacc, masks
from concourse.bass_utils import run_bass_kernel_spmd

NCORES = 8
BATCH = 32768
BPC = BATCH // NCORES  # 4096 rows per core
OBS = 256
HID = 512
NQP = 16  # q_u (4) + p_u (4) + 8 zero-pad cols (step%16==0 for DoubleRow)
BT = 512  # batch tile (matmul moving free dim)
NT = BPC // BT  # 8 batch tiles per core
LR = 0.01
NWARM = 5  # dummy matmuls to span the ~3.4us HAM warm-up window
F32 = mybir.dt.float32
FP8 = mybir.dt.float8e4
BF16 = mybir.dt.bfloat16
DR = mybir.MatmulPerfMode.DoubleRow

_CACHE = {}


def _build_nc(zero_bias):
    nc = bacc.Bacc(
        trn_type="TRN2", target_bir_lowering=False, debug=False, num_devices=NCORES
    )
    obsT = nc.declare_dram_parameter("obsT", [NT, 128, 2, BT], FP8, isOutput=False).ap()
    u0 = nc.declare_dram_parameter("u0", [NT // 2, 128, 4, 2, 4], F32, isOutput=False).ap()
    w1 = nc.declare_dram_parameter("w1", [128, 2, HID], FP8, isOutput=False).ap()
    w2 = nc.declare_dram_parameter("w2", [128, 4, HID], FP8, isOutput=False).ap()
    w3 = nc.declare_dram_parameter("w3", [128, 4, 64], FP8, isOutput=False).ap()
    b1 = nc.declare_dram_parameter("b1", [128, 4], F32, isOutput=False).ap()
    b2 = nc.declare_dram_parameter("b2", [128, 4], F32, isOutput=False).ap()
    b3 = nc.declare_dram_parameter("b3", [32, 1], F32, isOutput=False).ap()
    idm = nc.declare_dram_parameter("idm", [32, 32], F32, isOutput=False).ap()
    uo = nc.declare_dram_parameter("uo", [NT // 2, 128, 4, 2, 4], F32, isOutput=True).ap()

    AF = mybir.ActivationFunctionType
    ALU = mybir.AluOpType

    with tile.TileContext(nc) as tc:
        from contextlib import ExitStack

        with ExitStack() as ctx:
            singles = ctx.enter_context(tc.tile_pool(name="singles", bufs=1))
            p_obs = ctx.enter_context(tc.tile_pool(name="obs", bufs=4))
            p_y1 = ctx.enter_context(tc.tile_pool(name="y1", bufs=2))
            p_y2 = ctx.enter_context(tc.tile_pool(name="y2", bufs=2))
            p_qp = ctx.enter_context(tc.tile_pool(name="qp", bufs=2))
            p_cf = ctx.enter_context(tc.tile_pool(name="cf", bufs=2))
            # PSUM budget is 8 banks: y 3x2 + psq 2 (z3 borrows a y slot)
            pp_y = ctx.enter_context(tc.tile_pool(name="ppy", bufs=3, space="PSUM"))
            pp_q = ctx.enter_context(tc.tile_pool(name="ppq", bufs=2, space="PSUM"))

            # ---- PE warm-up: junk matmuls with no DMA dependency ----
            junk = singles.tile([128, 2, BT], FP8)
            nc.vector.memset(junk, 1.0)
            wps = pp_y.tile([128, 2, BT], F32, name="wps", tag="y")
            for i in range(NWARM):
                nc.tensor.matmul(
                    wps[:, i % 2, :],
                    junk[:, 0:2, 0:128],
                    junk,
                    start=True,
                    stop=True,
                    perf_mode=DR,
                )

            # ---- one-time: weights (already fp8/scaled on host), biases.
            # w1 goes on the SP ring (needed first, with pair-0 obs); the
            # rest ride the ACT HWDGE ring so they don't delay pair 0.
            w1s = singles.tile([128, 2, HID], FP8)
            nc.scalar.dma_start(out=w1s, in_=w1)
            w2s = singles.tile([128, 4, HID], FP8)
            nc.scalar.dma_start(out=w2s, in_=w2)
            w3s = singles.tile([128, 4, 64], FP8)
            nc.scalar.dma_start(out=w3s, in_=w3)
            b1s = singles.tile([128, 4], F32)
            nc.scalar.dma_start(out=b1s, in_=b1)
            b2s = singles.tile([128, 4], F32)
            nc.scalar.dma_start(out=b2s, in_=b2)
            b3s = singles.tile([32, 1], F32)
            nc.scalar.dma_start(out=b3s, in_=b3)
            # identity for the PE qp transposes
            ids = singles.tile([32, 32], F32)
            nc.scalar.dma_start(out=ids, in_=idm)




            def drain2(dst, src, bias_sb, mp, on_act, split=False):
                # dst [128, 2, 512] fp8 <- relu(src [128, 2, 512] psum + bias)
                # bias is per (partition, m-chunk); engine bias operands are
                # per-partition only, so nonzero bias needs per-chunk drains.
                if zero_bias:
                    if split:
                        # latency-critical: one bank per engine, in parallel
                        nc.scalar.activation(
                            out=dst[:, 0, :],
                            in_=src[:, 0, :],
                            func=AF.Relu,
                            bias=0.0,
                            scale=1.0,
                        )
                        nc.vector.tensor_scalar(
                            dst[:, 1, :], src[:, 1, :], 0.0, None, ALU.max
                        )
                    elif on_act:
                        nc.scalar.activation(
                            out=dst, in_=src, func=AF.Relu, bias=0.0, scale=1.0
                        )
                    else:
                        nc.vector.tensor_scalar(dst, src, 0.0, None, ALU.max)
                else:
                    for mi in range(2):
                        b = bias_sb[:, 2 * mp + mi : 2 * mp + mi + 1]
                        if on_act:
                            nc.scalar.activation(
                                out=dst[:, mi, :],
                                in_=src[:, mi, :],
                                func=AF.Relu,
                                bias=b,
                                scale=1.0,
                            )
                        else:
                            nc.vector.tensor_scalar(
                                dst[:, mi, :], src[:, mi, :], b, 0.0, ALU.add, ALU.max
                            )

            def tail(g, qpT, u0b):
                # transpose to batch-major + closed form + store, for pair g.
                # Emitted AFTER pair g+1's layer-1 matmuls so the PE never
                # idles waiting on pair g's sigmoid at the pair boundary.
                eng = nc.vector if g == NT // 2 - 1 else nc.gpsimd
                psq4 = pp_q.tile([128, 4, 32], F32, tag="psq")
                for c in range(4):
                    nc.tensor.transpose(
                        psq4[:, c, :], qpT[:, c * 128 : (c + 1) * 128], ids[:]
                    )
                psq = psq4.rearrange("x c (u j) -> x c u j", u=2)
                q = psq[:, :, :, 0:4]
                p = psq[:, :, :, 4:8]
                # DVE reads PSUM: a = 1-2*lr*q, rq = 1/q, r = p/q
                SH = [128, 4, 2, 4]
                a = p_cf.tile(SH, F32, tag="a")
                nc.vector.tensor_scalar(a, q, -2.0 * LR, 1.0, ALU.mult, ALU.add)
                rq = p_cf.tile(SH, F32, tag="rq")
                nc.vector.reciprocal(rq, q)
                r = p_cf.tile(SH, F32, tag="r")
                nc.vector.tensor_mul(r, p, rq)
                # A = a^100 and the final update (GPSIMD; DVE for last pair)
                GM = eng.tensor_mul
                a2 = p_cf.tile(SH, F32, tag="a2")
                GM(a2, a, a)
                a4 = p_cf.tile(SH, F32, tag="a4")
                GM(a4, a2, a2)
                a8 = p_cf.tile(SH, F32, tag="a8")
                GM(a8, a4, a4)
                a16 = p_cf.tile(SH, F32, tag="a16")
                GM(a16, a8, a8)
                a32 = p_cf.tile(SH, F32, tag="a32")
                GM(a32, a16, a16)
                a64 = p_cf.tile(SH, F32, tag="a64")
                GM(a64, a32, a32)
                a96 = p_cf.tile(SH, F32, tag="a96")
                GM(a96, a64, a32)
                A = p_cf.tile(SH, F32, tag="A")
                GM(A, a96, a4)
                n1 = p_cf.tile(SH, F32, tag="n1")  # 0.5*(1-A)
                eng.tensor_scalar(n1, A, -0.5, 0.5, ALU.mult, ALU.add)
                tt = p_cf.tile(SH, F32, tag="tt")
                GM(tt, r, n1)
                mm = p_cf.tile(SH, F32, tag="mm")
                GM(mm, A, u0b)
                uob = p_cf.tile(SH, F32, tag="uob")
                eng.tensor_sub(uob, mm, tt)
                nc.sync.dma_start(out=uo[g], in_=uob)

            pend = None
            for g in range(NT // 2):
                ts = (2 * g, 2 * g + 1)
                obsb = {}
                for t in ts:
                    ob = p_obs.tile([128, 2, BT], FP8, name="ob", tag="obs")
                    nc.sync.dma_start(out=ob, in_=obsT[t])
                    obsb[t] = ob
                u0b = p_cf.tile([128, 4, 2, 4], F32, tag="u0b")
                nc.sync.dma_start(out=u0b, in_=u0[g])

                # layer 1: psum = z1 (scales folded on host); pair shares LDW;
                # psum groups span 2 banks so one drain moves 128x1024 values
                y1 = {
                    t: p_y1.tile(
                        [128, 4, HID], FP8, name=f"y1_{t % 2}", tag=f"y1_{t % 2}"
                    )
                    for t in ts
                }
                for mp in range(2):
                    ps1 = {
                        t: pp_y.tile([128, 2, BT], F32, name="ps1", tag="y") for t in ts
                    }
                    for mi in range(2):
                        m = 2 * mp + mi
                        for t in ts:
                            nc.tensor.matmul(
                                ps1[t][:, mi, :],
                                w1s[:, 0:2, m * 128 : (m + 1) * 128],
                                obsb[t],
                                start=True,
                                stop=True,
                                perf_mode=DR,
                            )
                    for i, t in enumerate(ts):
                        drain2(
                            y1[t][:, 2 * mp : 2 * mp + 2, :],
                            ps1[t],
                            b1s,
                            mp,
                            (mp + i) % 2 == 0,
                            split=True,
                        )

                if pend is not None:
                    tail(*pend)
                    pend = None

                # layer 2: psum = 64*z2; drain y2' = relu(psum + 64*b2) = 64*y2
                y2 = {
                    t: p_y2.tile(
                        [128, 4, HID], FP8, name=f"y2_{t % 2}", tag=f"y2_{t % 2}"
                    )
                    for t in ts
                }
                for mp in range(2):
                    ps2 = {
                        t: pp_y.tile([128, 2, BT], F32, name="ps2", tag="y") for t in ts
                    }
                    for mi in range(2):
                        m = 2 * mp + mi
                        for kc in range(2):
                            for t in ts:
                                nc.tensor.matmul(
                                    ps2[t][:, mi, :],
                                    w2s[
                                        :, 2 * kc : 2 * kc + 2, m * 128 : (m + 1) * 128
                                    ],
                                    y1[t][:, 2 * kc : 2 * kc + 2, :],
                                    start=(kc == 0),
                                    stop=(kc == 1),
                                    perf_mode=DR,
                                )
                    for i, t in enumerate(ts):
                        drain2(
                            y2[t][:, 2 * mp : 2 * mp + 2, :],
                            ps2[t],
                            b2s,
                            mp,
                            (mp + i) % 2 == 1,
                        )

                # layer 3: psum = 1024*z3 for both tiles in ONE bank (t0 at
                # partitions 0:16, t1 at 32:48 -> concurrent col-groups), then
                # fused bias+sigmoid on ACT into a pair-merged qpT [32, 512]
                z3y = pp_y.tile([128, 2, BT], F32, name="z3y", tag="y")
                for i, t in enumerate(ts):
                    for kc in range(2):
                        nc.tensor.matmul(
                            z3y[0:32, 0, :],
                            w3s[:, 2 * kc : 2 * kc + 2, 32 * i : 32 * i + 32],
                            y2[t][:, 2 * kc : 2 * kc + 2, :],
                            start=(i == 0 and kc == 0),
                            stop=(i == 1 and kc == 1),
                            perf_mode=DR,
                        )
                qpT = p_qp.tile([32, BT], F32, tag="qpT")
                nc.scalar.activation(
                    out=qpT,
                    in_=z3y[0:32, 0, :],
                    func=AF.Sigmoid,
                    bias=b3s[:, 0:1],
                    scale=1.0 / 1024.0,
                )
                pend = (g, qpT, u0b)
            tail(*pend)
    nc.finalize()
    return nc


def _get_nc(zero_bias):
    key = ("nc", zero_bias)
    if key not in _CACHE:
        _CACHE[key] = _build_nc(zero_bias)
    return _CACHE[key]


FP8NP = ml_dtypes.float8_e4m3  # TRN float8e4: bias 7, max normal +-240


def _to_fp8(x):
    return np.ascontiguousarray(np.clip(x, -240.0, 240.0)).astype(FP8NP)


def kernel(obs, x_init, u_init, W1, b1, W2, b2, W3, b3):
    obs = np.asarray(obs, dtype=np.float32)
    u_init = np.ascontiguousarray(np.asarray(u_init, dtype=np.float32))
    W1 = np.asarray(W1, dtype=np.float32)
    W2 = np.asarray(W2, dtype=np.float32)
    W3 = np.asarray(W3, dtype=np.float32)
    b1 = np.asarray(b1, dtype=np.float32)
    b2 = np.asarray(b2, dtype=np.float32)
    b3 = np.asarray(b3, dtype=np.float32)

    # weights with fp8 scale folding (see module docstring)
    w1h = _to_fp8((8.0 * W1).reshape(2, 128, HID).transpose(1, 0, 2))
    w2h = _to_fp8((64.0 * W2).reshape(4, 128, HID).transpose(1, 0, 2))
    w3u = np.zeros((HID, 64), dtype=np.float32)
    w3u[:, 0:4] = 16.0 * W3[:, 12:16]  # q_u (tile 0 of each pair)
    w3u[:, 4:8] = 16.0 * W3[:, 28:32]  # p_u
    w3u[:, 48:52] = 16.0 * W3[:, 12:16]  # q_u (tile 1 -> psum rows 16:32)
    w3u[:, 52:56] = 16.0 * W3[:, 28:32]  # p_u
    w3h = _to_fp8(w3u.reshape(4, 128, 64).transpose(1, 0, 2))
    b1p = np.ascontiguousarray(b1.reshape(4, 128).T)
    b2p = np.ascontiguousarray(64.0 * b2.reshape(4, 128).T)
    b3p = np.zeros((32, 1), dtype=np.float32)
    b3p[0:4, 0] = b3[12:16]
    b3p[4:8, 0] = b3[28:32]
    b3p[16:20, 0] = b3[12:16]
    b3p[20:24, 0] = b3[28:32]
    idp = np.eye(32, dtype=np.float32)

    zero_bias = bool(np.all(b1 == 0.0) and np.all(b2 == 0.0))
    nc = _get_nc(zero_bias)
    in_maps = []
    for i in range(NCORES):
        oc = obs[i * BPC : (i + 1) * BPC]  # [4096, 256]
        # [t, p, kc, n] = obs[t*512+n, kc*128+p] / 8
        obsT = _to_fp8(oc.reshape(NT, BT, 2, 128).transpose(0, 3, 2, 1) / 8.0)
        # u0[g, p, c, u, j] = u_init[(2g+u)*512 + c*128 + p, j]
        u0c = u_init[i * BPC : (i + 1) * BPC].reshape(NT // 2, 2, 4, 128, 4)
        u0c = np.ascontiguousarray(u0c.transpose(0, 3, 2, 1, 4))
        in_maps.append(
            {
                "obsT": obsT,
                "u0": u0c,
                "w1": w1h,
                "w2": w2h,
                "w3": w3h,
                "b1": b1p,
                "b2": b2p,
                "b3": b3p,
                "idm": idp,
            }
        )
    import os

    kw = {}
    if os.environ.get("BASSK_TRACE"):
        kw = {"trace": True, "tmpdir": os.environ.get("BASSK_TRACE_DIR") or None}
    res = run_bass_kernel_spmd(nc, in_maps, list(range(NCORES)), **kw)
    _CACHE["last_result"] = res
    outs = []
    for i in range(NCORES):
        arr = np.asarray(res.results[i]["uo"])  # [g, p, c, u, j]
        outs.append(arr.transpose(0, 3, 2, 1, 4).reshape(BPC, 4))
    return np.concatenate(outs, axis=0).astype(np.float32)


# revision 56
# speedup vs baseline: 1.0408x; 1.0408x over previous
"""Trainium2 Bass kernel for nn_MPCActor: MLP (256->512->512->8-useful-cols,
relu/relu/sigmoid) followed by the closed-form equivalent of 100 SGD steps on
u (u_N = A*u0 - 0.5*(p/q)*(1-A), A = (1-2*lr*q)^100).

Data parallel over 8 NeuronCores: batch 32768 -> 4096 rows per core, weights
replicated. All matmuls run in fp8(e4m3) with perf_mode=DoubleRow (K=256 per
instruction, 2x FLOP throughput at the same 1 col/cycle streaming rate). The
obs transpose, weight layouts, fp8 casts and scale folding are done on the
host:
  obsT = obs.T/8 (fp8)      W1h = 8*W1 (fp8)   -> psum1 = z1 exactly
  y1   = relu(z1+b1) (fp8)  W2h = 64*W2 (fp8)  -> psum2 = 64*z2
  y2'  = relu(psum2+64*b2) = 64*y2 (fp8)
  W3h  = 16*W3[:, useful] padded to 16 cols    -> psum3 = 1024*z3
  qpT  = sigmoid(psum3/1024 + b3)  (ACT, feature-major)

PSUM drains are 1 elem/cycle on ACT/DVE (and DMA has no PSUM route), so PSUM
y-tiles span two banks ([128,2,512]) and each drain moves 128x1024 values in
one instruction; three such groups rotate so both drain engines stay busy
while the PE fills the third. The pair's two z3 results share one PSUM bank
at partition offsets 0/32 (concurrent col-groups via tile_position), the
transposed qp lands in one more bank, and the closed-form runs per pair on
the otherwise idle GPSIMD. A short burst of dummy matmuls at t=0 keeps the
PE busy through the HAM warm-up window so real matmuls run at 2.4 GHz.
"""

import numpy as np
import ml_dtypes

import concourse.bass as bass
import concourse.mybir as mybir
import concourse.tile as tile
from concourse import bacc, masks
from concourse.bass_utils import run_bass_kernel_spmd

NCORES = 8
BATCH = 32768
BPC = BATCH // NCORES  # 4096 rows per core
OBS = 256
HID = 512
NQP = 16  # q_u (4) + p_u (4) + 8 zero-pad cols (step%16==0 for DoubleRow)
BT = 512  # batch tile (matmul moving free dim)
NT = BPC // BT  # 8 batch tiles per core
LR = 0.01
NWARM = 5  # dummy matmuls to span the ~3.4us HAM warm-up window
F32 = mybir.dt.float32
FP8 = mybir.dt.float8e4
BF16 = mybir.dt.bfloat16
DR = mybir.MatmulPerfMode.DoubleRow

_CACHE = {}


def _build_nc(zero_bias):
    nc = bacc.Bacc(
        trn_type="TRN2", target_bir_lowering=False, debug=False, num_devices=NCORES
    )
    obsT = nc.declare_dram_parameter("obsT", [NT, 128, 2, BT], FP8, isOutput=False).ap()
    u0 = nc.declare_dram_parameter("u0", [NT // 2, 128, 4, 2, 4], F32, isOutput=False).ap()
    w1 = nc.declare_dram_parameter("w1", [128, 2, HID], FP8, isOutput=False).ap()
    w2 = nc.declare_dram_parameter("w2", [128, 4, HID], FP8, isOutput=False).ap()
    w3 = nc.declare_dram_parameter("w3", [128, 4, 64], FP8, isOutput=False).ap()
    b1 = nc.declare_dram_parameter("b1", [128, 4], F32, isOutput=False).ap()
    b2 = nc.declare_dram_parameter("b2", [128, 4], F32, isOutput=False).ap()
    b3 = nc.declare_dram_parameter("b3", [32, 1], F32, isOutput=False).ap()
    idm = nc.declare_dram_parameter("idm", [32, 32], F32, isOutput=False).ap()
    uo = nc.declare_dram_parameter("uo", [NT // 2, 128, 4, 2, 4], F32, isOutput=True).ap()

    AF = mybir.ActivationFunctionType
    ALU = mybir.AluOpType

    with tile.TileContext(nc) as tc:
        from contextlib import ExitStack

        with ExitStack() as ctx:
            singles = ctx.enter_context(tc.tile_pool(name="singles", bufs=1))
            p_obs = ctx.enter_context(tc.tile_pool(name="obs", bufs=4))
            p_y1 = ctx.enter_context(tc.tile_pool(name="y1", bufs=2))
            p_y2 = ctx.enter_context(tc.tile_pool(name="y2", bufs=2))
            p_qp = ctx.enter_context(tc.tile_pool(name="qp", bufs=2))
            p_cf = ctx.enter_context(tc.tile_pool(name="cf", bufs=2))
            # PSUM budget is 8 banks: y 3x2 + psq 2 (z3 borrows a y slot)
            pp_y = ctx.enter_context(tc.tile_pool(name="ppy", bufs=3, space="PSUM"))
            pp_q = ctx.enter_context(tc.tile_pool(name="ppq", bufs=2, space="PSUM"))

            # ---- PE warm-up: junk matmuls with no DMA dependency ----
            junk = singles.tile([128, 2, BT], FP8)
            nc.vector.memset(junk, 1.0)
            wps = pp_y.tile([128, 2, BT], F32, name="wps", tag="y")
            for i in range(NWARM):
                nc.tensor.matmul(
                    wps[:, i % 2, :],
                    junk[:, 0:2, 0:128],
                    junk,
                    start=True,
                    stop=True,
                    perf_mode=DR,
                )

            # ---- one-time: weights (already fp8/scaled on host), biases.
            # w1 goes on the SP ring (needed first, with pair-0 obs); the
            # rest ride the ACT HWDGE ring so they don't delay pair 0.
            w1s = singles.tile([128, 2, HID], FP8)
            nc.scalar.dma_start(out=w1s, in_=w1)
            w2s = singles.tile([128, 4, HID], FP8)
            nc.scalar.dma_start(out=w2s, in_=w2)
            w3s = singles.tile([128, 4, 64], FP8)
            nc.scalar.dma_start(out=w3s, in_=w3)
            b1s = singles.tile([128, 4], F32)
            nc.scalar.dma_start(out=b1s, in_=b1)
            b2s = singles.tile([128, 4], F32)
            nc.scalar.dma_start(out=b2s, in_=b2)
            b3s = singles.tile([32, 1], F32)
            nc.scalar.dma_start(out=b3s, in_=b3)
            # identity for the PE qp transposes
            ids = singles.tile([32, 32], F32)
            nc.scalar.dma_start(out=ids, in_=idm)




            def drain2(dst, src, bias_sb, mp, on_act, split=False):
                # dst [128, 2, 512] fp8 <- relu(src [128, 2, 512] psum + bias)
                # bias is per (partition, m-chunk); engine bias operands are
                # per-partition only, so nonzero bias needs per-chunk drains.
                if zero_bias:
                    if on_act:
                        nc.scalar.activation(
                            out=dst, in_=src, func=AF.Relu, bias=0.0, scale=1.0
                        )
                    else:
                        nc.vector.tensor_scalar(dst, src, 0.0, None, ALU.max)
                else:
                    for mi in range(2):
                        b = bias_sb[:, 2 * mp + mi : 2 * mp + mi + 1]
                        if on_act:
                            nc.scalar.activation(
                                out=dst[:, mi, :],
                                in_=src[:, mi, :],
                                func=AF.Relu,
                                bias=b,
                                scale=1.0,
                            )
                        else:
                            nc.vector.tensor_scalar(
                                dst[:, mi, :], src[:, mi, :], b, 0.0, ALU.add, ALU.max
                            )

            def tail(g, qpT, u0b):
                # transpose to batch-major + closed form + store, for pair g.
                # Emitted AFTER pair g+1's layer-1 matmuls so the PE never
                # idles waiting on pair g's sigmoid at the pair boundary.
                eng = nc.vector if g == NT // 2 - 1 else nc.gpsimd
                psq4 = pp_q.tile([128, 4, 32], F32, tag="psq")
                for c in range(4):
                    nc.tensor.transpose(
                        psq4[:, c, :], qpT[:, c * 128 : (c + 1) * 128], ids[:]
                    )
                psq = psq4.rearrange("x c (u j) -> x c u j", u=2)
                q = psq[:, :, :, 0:4]
                p = psq[:, :, :, 4:8]
                # DVE reads PSUM: a = 1-2*lr*q, rq = 1/q, r = p/q
                SH = [128, 4, 2, 4]
                a = p_cf.tile(SH, F32, tag="a")
                nc.vector.tensor_scalar(a, q, -2.0 * LR, 1.0, ALU.mult, ALU.add)
                rq = p_cf.tile(SH, F32, tag="rq")
                nc.vector.reciprocal(rq, q)
                r = p_cf.tile(SH, F32, tag="r")
                nc.vector.tensor_mul(r, p, rq)
                # A = a^100 and the final update (GPSIMD; DVE for last pair)
                GM = eng.tensor_mul
                a2 = p_cf.tile(SH, F32, tag="a2")
                GM(a2, a, a)
                a4 = p_cf.tile(SH, F32, tag="a4")
                GM(a4, a2, a2)
                a8 = p_cf.tile(SH, F32, tag="a8")
                GM(a8, a4, a4)
                a16 = p_cf.tile(SH, F32, tag="a16")
                GM(a16, a8, a8)
                a32 = p_cf.tile(SH, F32, tag="a32")
                GM(a32, a16, a16)
                a64 = p_cf.tile(SH, F32, tag="a64")
                GM(a64, a32, a32)
                a96 = p_cf.tile(SH, F32, tag="a96")
                GM(a96, a64, a32)
                A = p_cf.tile(SH, F32, tag="A")
                GM(A, a96, a4)
                n1 = p_cf.tile(SH, F32, tag="n1")  # 0.5*(1-A)
                eng.tensor_scalar(n1, A, -0.5, 0.5, ALU.mult, ALU.add)
                tt = p_cf.tile(SH, F32, tag="tt")
                GM(tt, r, n1)
                mm = p_cf.tile(SH, F32, tag="mm")
                GM(mm, A, u0b)
                uob = p_cf.tile(SH, F32, tag="uob")
                eng.tensor_sub(uob, mm, tt)
                nc.sync.dma_start(out=uo[g], in_=uob)

            pend = None
            for g in range(NT // 2):
                ts = (2 * g, 2 * g + 1)
                obsb = {}
                for t in ts:
                    ob = p_obs.tile([128, 2, BT], FP8, name="ob", tag="obs")
                    nc.sync.dma_start(out=ob, in_=obsT[t])
                    obsb[t] = ob
                u0b = p_cf.tile([128, 4, 2, 4], F32, tag="u0b")
                nc.sync.dma_start(out=u0b, in_=u0[g])

                # layer 1: psum = z1 (scales folded on host); pair shares LDW;
                # psum groups span 2 banks so one drain moves 128x1024 values
                y1 = {
                    t: p_y1.tile(
                        [128, 4, HID], FP8, name=f"y1_{t % 2}", tag=f"y1_{t % 2}"
                    )
                    for t in ts
                }
                for mp in range(2):
                    ps1 = {
                        t: pp_y.tile([128, 2, BT], F32, name="ps1", tag="y") for t in ts
                    }
                    for mi in range(2):
                        m = 2 * mp + mi
                        for t in ts:
                            nc.tensor.matmul(
                                ps1[t][:, mi, :],
                                w1s[:, 0:2, m * 128 : (m + 1) * 128],
                                obsb[t],
                                start=True,
                                stop=True,
                                perf_mode=DR,
                            )
                    for i, t in enumerate(ts):
                        drain2(
                            y1[t][:, 2 * mp : 2 * mp + 2, :],
                            ps1[t],
                            b1s,
                            mp,
                            (mp + i) % 2 == 0,
                        )

                if pend is not None:
                    tail(*pend)
                    pend = None

                # layer 2: psum = 64*z2; drain y2' = relu(psum + 64*b2) = 64*y2
                y2 = {
                    t: p_y2.tile(
                        [128, 4, HID], FP8, name=f"y2_{t % 2}", tag=f"y2_{t % 2}"
                    )
                    for t in ts
                }
                for mp in range(2):
                    ps2 = {
                        t: pp_y.tile([128, 2, BT], F32, name="ps2", tag="y") for t in ts
                    }
                    for mi in range(2):
                        m = 2 * mp + mi
                        for kc in range(2):
                            for t in ts:
                                nc.tensor.matmul(
                                    ps2[t][:, mi, :],
                                    w2s[
                                        :, 2 * kc : 2 * kc + 2, m * 128 : (m + 1) * 128
                                    ],
                                    y1[t][:, 2 * kc : 2 * kc + 2, :],
                                    start=(kc == 0),
                                    stop=(kc == 1),
                                    perf_mode=DR,
                                )
                    for i, t in enumerate(ts):
                        drain2(
                            y2[t][:, 2 * mp : 2 * mp + 2, :],
                            ps2[t],
                            b2s,
                            mp,
                            (mp + i) % 2 == 1,
                        )

                # layer 3: psum = 1024*z3 for both tiles in ONE bank (t0 at
                # partitions 0:16, t1 at 32:48 -> concurrent col-groups), then
                # fused bias+sigmoid on ACT into a pair-merged qpT [32, 512]
                z3y = pp_y.tile([128, 2, BT], F32, name="z3y", tag="y")
                for i, t in enumerate(ts):
                    for kc in range(2):
                        nc.tensor.matmul(
                            z3y[0:32, 0, :],
                            w3s[:, 2 * kc : 2 * kc + 2, 32 * i : 32 * i + 32],
                            y2[t][:, 2 * kc : 2 * kc + 2, :],
                            start=(i == 0 and kc == 0),
                            stop=(i == 1 and kc == 1),
                            perf_mode=DR,
                        )
                qpT = p_qp.tile([32, BT], F32, tag="qpT")
                nc.scalar.activation(
                    out=qpT,
                    in_=z3y[0:32, 0, :],
                    func=AF.Sigmoid,
                    bias=b3s[:, 0:1],
                    scale=1.0 / 1024.0,
                )
                pend = (g, qpT, u0b)
            tail(*pend)
    nc.finalize()
    return nc


def _get_nc(zero_bias):
    key = ("nc", zero_bias)
    if key not in _CACHE:
        _CACHE[key] = _build_nc(zero_bias)
    return _CACHE[key]


FP8NP = ml_dtypes.float8_e4m3  # TRN float8e4: bias 7, max normal +-240


def _to_fp8(x):
    return np.ascontiguousarray(np.clip(x, -240.0, 240.0)).astype(FP8NP)


def kernel(obs, x_init, u_init, W1, b1, W2, b2, W3, b3):
    obs = np.asarray(obs, dtype=np.float32)
    u_init = np.ascontiguousarray(np.asarray(u_init, dtype=np.float32))
    W1 = np.asarray(W1, dtype=np.float32)
    W2 = np.asarray(W2, dtype=np.float32)
    W3 = np.asarray(W3, dtype=np.float32)
    b1 = np.asarray(b1, dtype=np.float32)
    b2 = np.asarray(b2, dtype=np.float32)
    b3 = np.asarray(b3, dtype=np.float32)

    # weights with fp8 scale folding (see module docstring)
    w1h = _to_fp8((8.0 * W1).reshape(2, 128, HID).transpose(1, 0, 2))
    w2h = _to_fp8((64.0 * W2).reshape(4, 128, HID).transpose(1, 0, 2))
    w3u = np.zeros((HID, 64), dtype=np.float32)
    w3u[:, 0:4] = 16.0 * W3[:, 12:16]  # q_u (tile 0 of each pair)
    w3u[:, 4:8] = 16.0 * W3[:, 28:32]  # p_u
    w3u[:, 48:52] = 16.0 * W3[:, 12:16]  # q_u (tile 1 -> psum rows 16:32)
    w3u[:, 52:56] = 16.0 * W3[:, 28:32]  # p_u
    w3h = _to_fp8(w3u.reshape(4, 128, 64).transpose(1, 0, 2))
    b1p = np.ascontiguousarray(b1.reshape(4, 128).T)
    b2p = np.ascontiguousarray(64.0 * b2.reshape(4, 128).T)
    b3p = np.zeros((32, 1), dtype=np.float32)
    b3p[0:4, 0] = b3[12:16]
    b3p[4:8, 0] = b3[28:32]
    b3p[16:20, 0] = b3[12:16]
    b3p[20:24, 0] = b3[28:32]
    idp = np.eye(32, dtype=np.float32)

    zero_bias = bool(np.all(b1 == 0.0) and np.all(b2 == 0.0))
    nc = _get_nc(zero_bias)
    in_maps = []
    for i in range(NCORES):
        oc = obs[i * BPC : (i + 1) * BPC]  # [4096, 256]
        # [t, p, kc, n] = obs[t*512+n, kc*128+p] / 8
        obsT = _to_fp8(oc.reshape(NT, BT, 2, 128).transpose(0, 3, 2, 1) / 8.0)
        # u0[g, p, c, u, j] = u_init[(2g+u)*512 + c*128 + p, j]
        u0c = u_init[i * BPC : (i + 1) * BPC].reshape(NT // 2, 2, 4, 128, 4)
        u0c = np.ascontiguousarray(u0c.transpose(0, 3, 2, 1, 4))
        in_maps.append(
            {
                "obsT": obsT,
                "u0": u0c,
                "w1": w1h,
                "w2": w2h,
                "w3": w3h,
                "b1": b1p,
                "b2": b2p,
                "b3": b3p,
                "idm": idp,
            }
        )
    import os

    kw = {}
    if os.environ.get("BASSK_TRACE"):
        kw = {"trace": True, "tmpdir": os.environ.get("BASSK_TRACE_DIR") or None}
    res = run_bass_kernel_spmd(nc, in_maps, list(range(NCORES)), **kw)
    _CACHE["last_result"] = res
    outs = []
    for i in range(NCORES):
        arr = np.asarray(res.results[i]["uo"])  # [g, p, c, u, j]
        outs.append(arr.transpose(0, 3, 2, 1, 4).reshape(BPC, 4))
    return np.concatenate(outs, axis=0).astype(np.float32)
